# revision 35
# baseline (speedup 1.0000x reference)
"""Trainium2 Bass kernel for nn_MultiHeadAttention_28260884808093.

MHA without QKV projections: heads formed by reshaping inputs directly,
scores scaled by 1/head_dim (not sqrt), softmax, attn@V, then fc_out.

Sharding: 8 cores = (batch, seq-half). Each core owns a disjoint
[1024, 1024] slice of the final output, so no device collectives are
needed (fc_out mixes head dims, not tokens). Host pre-transposes
q/k/fc_w so every matmul contraction lands on the partition axis.

Loop nest: pair j (8) -> m-half mh (2) -> key chunk c (16), flattened
into 256 substeps. Per substep, per engine:
  PE   : QK^T for both heads (row-tiled, concurrent) + the PV
         accumulation lagged PV_LAG substeps (so the PE never waits on
         the exp) -> stays HAM-warm at 2.4 GHz.
  ACT  : exp for the even head (exact, table spline).
  DVE  : exp for the odd head via a Schraudolph bf16 bit-trick
         (one tensor_scalar: i16 = s*128/(ln2*64) + (16256-c), bits
         reinterpreted as bf16). Softmax normalization cancels the
         common-mode error; keeping whole heads pure keeps the residual
         small (measured ~1.0e-2 vs the 2e-2 gate).
  GPSIMD: ONLY partition_broadcast (one library resident -- any second
         gpsimd op type causes ~5us LOAD_LIB swaps per call).
PV accumulators are [65, 512] (1 PSUM bank) per (pair, mh, head), Z
rides as a ones-column in V. Normalize = DVE reciprocal_approx_fast on
the Z row (PSUM src) + gpsimd broadcast + DVE multiply out of PSUM.

All matmul weights are full 128x128 blocks (K zero-padded per head, V
zero-padded across columns): that enables the compiler's automatic
fast-weight-load path, which is what lets back-to-back N=512 bf16
matmuls issue at the 216ns streaming cadence instead of the ~380ns
isolated-matmul latency.
"""

import sys

sys.path.insert(0, "/opt/trn_rl_repo")

import ml_dtypes
import numpy as np
from contextlib import ExitStack

import concourse.bass as bass  # noqa: F401
import concourse.bacc as bacc
import concourse.tile as tile
from concourse import mybir
from concourse import bass_utils
from concourse.bass_utils import run_bass_kernel_spmd

B, S, D = 4, 2048, 1024
H, DH = 16, 64
N_CORES = 8
M = (B * S) // N_CORES  # 1024 query tokens per core
PAIRS = H // 2
NCH = S // 128  # 16 key chunks of 128
NSUB = PAIRS * 2 * NCH  # 256 substeps: (pair, m-half, chunk)
PV_LAG = 4  # substeps PV trails QK/exp

F32 = mybir.dt.float32
BF16 = mybir.dt.bfloat16
FP8 = mybir.dt.float8e4
I16 = mybir.dt.int16
NP_DT = ml_dtypes.bfloat16
NP_FP8 = ml_dtypes.float8_e4m3

# Schraudolph exp->bf16-bits constants (odd heads, DVE):
#   i16 = round(s * 2^7/(ln2*DH) + (127*2^7 - C)) ; bits = bf16
LN2 = float(np.log(2.0))
SCHRAU_C1 = 128.0 / (LN2 * DH)
SCHRAU_C = 8.0
SCHRAU_C2 = 127.0 * 128.0 - SCHRAU_C

# Pairs whose odd head also runs on ACT (engine balance: DVE carries the
# normalize ops, so it takes 7 of the 8 odd heads, not 8).
ACT_B_PAIRS = frozenset({3})


def _sub_idx(u):
    return u // 32, (u // 16) % 2, u % 16  # pair, m-half, chunk


def _mha_body(ctx, tc, qT, kT, v, fw, fb, out):
    nc = tc.nc
    sb = ctx.enter_context(tc.tile_pool(name="sb", bufs=1))
    ps = ctx.enter_context(tc.tile_pool(name="ps", bufs=1, space="PSUM"))

    # PE warm-up burst: dense dummy matmuls while the first DMAs land,
    # so the HAM clock-gate flips to 2.4 GHz before the real work starts.
    wtile = sb.tile([128, 512], BF16, name="wtile", tag="wtile")
    nc.vector.memset(wtile[:], 0.0)
    for wi in range(20):
        wps = ps.tile(
            [128, 512], F32, name="wps", tag=("stA" if wi % 2 == 0 else "stB"), bufs=2
        )
        nc.tensor.matmul(
            wps[:], lhsT=wtile[:, 0:128], rhs=wtile[:], start=True, stop=True
        )

    # ---- resident tensors (pair-0 critical loads first; fc weights at
    # the end so they don't delay the pipeline ramp) ----
    qt_sb, fw_sb, attn = [], [], []
    for j in range(8):
        qt = sb.tile([128, M], BF16, name=f"qt{j}", tag=f"qt{j}")
        nc.sync.dma_start(out=qt[:], in_=qT[j * 128 : (j + 1) * 128, :])
        qt_sb.append(qt)
        at = sb.tile([128, M], BF16, name=f"attn{j}", tag=f"attn{j}")
        attn.append(at)

    # v as [p, chunk, d] so one DMA per head loads all 16 chunks
    v_pcd = v.rearrange("(c p) d -> p c d", p=128)

    # K weights zero-padded to full 128 contraction partitions (full-
    # width LDWEIGHTS pipelines; narrow ones do not): head A in rows
    # 0:64 / zeros below, head B in rows 64:128 / zeros above.
    # Persistent rotating slots -> the zero halves are memset exactly once.
    ktA_slots, ktB_slots = [], []
    for s in range(3):
        ka = sb.tile([128, S], BF16, name=f"ktA{s}", tag=f"ktA{s}")
        ktA_slots.append(ka)
        kb = sb.tile([128, S], BF16, name=f"ktB{s}", tag=f"ktB{s}")
        ktB_slots.append(kb)

    # V weights at full 128 columns (enables the compiler's automatic
    # fast-weight-load): col 0 = ones (Z lands at PSUM partition 0,
    # 32-aligned for the custom reciprocal), cols 64:128 = v (PV rows
    # at partitions 64:128), cols 1:64 zero. Persistent rotating slots
    # so the constant columns are memset exactly once.
    va_slots, vb_slots = [], []
    for s in range(3):
        for nm, slots in (("va", va_slots), ("vb", vb_slots)):
            vt_ = sb.tile([128, NCH, 128], BF16, name=f"{nm}{s}", tag=f"{nm}{s}")
            slots.append(vt_)
    # constant-region fills AFTER the warm-up burst emission so the DVE
    # runs the wtile memset first and the burst starts at ~0.
    for s in range(3):
        nc.vector.memset(ktA_slots[s][64:128, :], 0.0)
        nc.vector.memset(ktB_slots[s][0:64, :], 0.0)
        for slots in (va_slots, vb_slots):
            nc.vector.memset(slots[s][:, :, 0:1], 1.0)
            nc.vector.memset(slots[s][:, :, 1:DH], 0.0)

    kt_t, va_t, vb_t = {}, {}, {}

    def issue_pair_loads(j):
        ka = ktA_slots[j % 3]
        nc.sync.dma_start(out=ka[0:64, :], in_=kT[j * 128 : j * 128 + 64, :])
        kb = ktB_slots[j % 3]
        nc.sync.dma_start(out=kb[64:128, :], in_=kT[j * 128 + 64 : (j + 1) * 128, :])
        kt_t[j] = (ka, kb)
        va = va_slots[j % 3]
        nc.sync.dma_start(
            out=va[:, :, DH:128], in_=v_pcd[:, :, (2 * j) * DH : (2 * j + 1) * DH]
        )
        va_t[j] = va
        vb = vb_slots[j % 3]
        nc.sync.dma_start(
            out=vb[:, :, DH:128], in_=v_pcd[:, :, (2 * j + 1) * DH : (2 * j + 2) * DH]
        )
        vb_t[j] = vb

    issue_pair_loads(0)

    # fc weights + bias, needed only in the fc phase
    for j in range(8):
        fwt = sb.tile([128, D], BF16, name=f"fw{j}", tag=f"fw{j}")
        nc.sync.dma_start(out=fwt[:], in_=fw[j * 128 : (j + 1) * 128, :])
        fw_sb.append(fwt)
    fb_sb = sb.tile([1, D], F32, name="fb_sb", tag="fb")
    nc.sync.dma_start(out=fb_sb[:], in_=fb[0:1, :])
    fbb = sb.tile([128, D], F32, name="fbb", tag="fbb")
    nc.gpsimd.partition_broadcast(fbb[:], fb_sb[:], channels=128)

    po_tiles = {}
    pa_hist, pb_hist = {}, {}

    def emit_qk_exp(u):
        j, mh, c = _sub_idx(u)
        if mh == 0 and c == 0 and j + 1 < PAIRS:
            issue_pair_loads(j + 1)
        if c == 0:
            oA = ps.tile([128, 512], F32, name="oA", tag="po", bufs=4)
            oB = ps.tile([128, 512], F32, name="oB", tag="po", bufs=4)
            po_tiles[(j, mh)] = (oA, oB)
        ka, kb = kt_t[j]
        ms = slice(mh * 512, (mh + 1) * 512)
        sA = ps.tile([128, 512], F32, name="sA", tag="stA", bufs=2)
        sB = ps.tile([128, 512], F32, name="sB", tag="stB", bufs=2)
        nc.tensor.matmul(
            sA[:],
            lhsT=ka[:, c * 128 : (c + 1) * 128],
            rhs=qt_sb[j][:, ms],
            start=True,
            stop=True,
        )
        nc.tensor.matmul(
            sB[:],
            lhsT=kb[:, c * 128 : (c + 1) * 128],
            rhs=qt_sb[j][:, ms],
            start=True,
            stop=True,
        )
        pa = sb.tile([128, 512], BF16, name="pa", tag="pa", bufs=8)
        nc.scalar.activation(
            out=pa[:],
            in_=sA[:],
            func=mybir.ActivationFunctionType.Exp,
            bias=0.0,
            scale=1.0 / DH,
        )
        if j in ACT_B_PAIRS:
            pb = sb.tile([128, 512], BF16, name="pbx", tag="pb", bufs=8)
            nc.scalar.activation(
                out=pb[:],
                in_=sB[:],
                func=mybir.ActivationFunctionType.Exp,
                bias=0.0,
                scale=1.0 / DH,
            )
        else:
            pb = sb.tile([128, 512], I16, name="pb", tag="pb", bufs=8)
            nc.vector.tensor_scalar(
                out=pb[:],
                in0=sB[:],
                scalar1=SCHRAU_C1,
                scalar2=SCHRAU_C2,
                op0=mybir.AluOpType.mult,
                op1=mybir.AluOpType.add,
            )
        pa_hist[u] = pa
        pb_hist[u] = pb

    def emit_pv(u):
        j, mh, c = _sub_idx(u)
        oA, oB = po_tiles[(j, mh)]
        pa = pa_hist.pop(u)
        pb = pb_hist.pop(u)
        nc.tensor.matmul(
            oA[:],
            lhsT=va_t[j][:, c, :],
            rhs=pa[:],
            start=(c == 0),
            stop=(c == NCH - 1),
        )
        rhs_b = pb[:] if pb.dtype == BF16 else pb[:].bitcast(BF16)
        nc.tensor.matmul(
            oB[:],
            lhsT=vb_t[j][:, c, :],
            rhs=rhs_b,
            start=(c == 0),
            stop=(c == NCH - 1),
        )

    def group_finish_ops(j, mh):
        """Normalize ops for group (j, mh), interleaved one per substep.
        recip + mul on DVE (PSUM source), broadcast on gpsimd."""
        oA, oB = po_tiles.pop((j, mh))
        ms = slice(mh * 512, (mh + 1) * 512)
        ops = []
        for h, oX in ((0, oA), (1, oB)):
            po = h * 64
            rz = sb.tile([1, 512], F32, name="rz", tag="rz", bufs=4)
            zbc = sb.tile([64, 512], F32, name="zbc", tag="zbc", bufs=4)

            def f_recip(oX=oX, rz=rz):
                nc.vector.reciprocal_approx_fast(out=rz[:], in_=oX[0:1, :])

            def f_bcast(zbc=zbc, rz=rz):
                nc.gpsimd.partition_broadcast(zbc[:], rz[:], channels=64)

            def f_mul(oX=oX, zbc=zbc, j=j, po=po, ms=ms):
                nc.vector.tensor_mul(
                    attn[j][po : po + 64, ms], oX[DH:128, :], zbc[:]
                )

            ops += [f_recip, f_bcast, f_mul]
        # release both PSUM slots early: A-ops then B-recip before muls
        return [ops[0], ops[3], ops[1], ops[4], ops[2], ops[5]]

    pending = []
    for u in range(NSUB + PV_LAG):
        if u < NSUB:
            emit_qk_exp(u)
        if u >= PV_LAG:
            up = u - PV_LAG
            emit_pv(up)
            jp, mhp, cp = _sub_idx(up)
            if cp == NCH - 1:
                pending.extend(group_finish_ops(jp, mhp))
        if u < NSUB:
            for f in pending[:2]:
                f()
            del pending[:2]
    for f in pending:
        f()

    # ---- fc_out: out[m, o] = attn_T.T @ fw + b ----
    for mi in range(8):
        for s2 in range(2):
            os_ = slice(s2 * 512, (s2 + 1) * 512)
            pf = ps.tile(
                [128, 512], F32, name="pf", tag=("stA" if s2 == 0 else "stB"), bufs=2
            )
            for jj in range(8):
                nc.tensor.matmul(
                    pf[:],
                    lhsT=attn[jj][:, mi * 128 : (mi + 1) * 128],
                    rhs=fw_sb[jj][:, os_],
                    start=(jj == 0),
                    stop=(jj == 7),
                )
            ob = sb.tile([128, 512], F32, name="ob", tag="ob", bufs=4)
            nc.vector.tensor_add(ob[:], pf[:], fbb[:, os_])
            nc.sync.dma_start(out=out[mi * 128 : (mi + 1) * 128, os_], in_=ob[:])


def build_module():
    nc = bacc.Bacc("TRN2", target_bir_lowering=False, debug=False, num_devices=N_CORES)
    qT = nc.dram_tensor("qT", [D, M], BF16, kind="ExternalInput")
    kT = nc.dram_tensor("kT", [D, S], BF16, kind="ExternalInput")
    v = nc.dram_tensor("v", [S, D], BF16, kind="ExternalInput")
    fw = nc.dram_tensor("fw", [D, D], BF16, kind="ExternalInput")
    fb = nc.dram_tensor("fb", [1, D], F32, kind="ExternalInput")
    out = nc.dram_tensor("out", [M, D], F32, kind="ExternalOutput")
    with tile.TileContext(nc) as tc:
        with ExitStack() as ctx:
            _mha_body(ctx, tc, qT.ap(), kT.ap(), v.ap(), fw.ap(), fb.ap(), out.ap())
    nc.compile()
    return nc


_NC_CACHE = None


def _get_module():
    global _NC_CACHE
    if _NC_CACHE is None:
        _NC_CACHE = build_module()
    return _NC_CACHE


def make_in_maps(query, key, value, fc_w, fc_b):
    fw_host = np.ascontiguousarray(fc_w.T).astype(NP_DT)
    fb_host = np.ascontiguousarray(np.asarray(fc_b, np.float32).reshape(1, D))
    in_maps = []
    kT_cache, v_cache = {}, {}
    for c in range(N_CORES):
        b, half = c // 2, c % 2
        if b not in kT_cache:
            kT_cache[b] = np.ascontiguousarray(key[b].T).astype(NP_DT)
            v_cache[b] = np.ascontiguousarray(value[b]).astype(NP_DT)
        qslice = query[b, half * M : (half + 1) * M, :]
        in_maps.append(
            {
                "qT": np.ascontiguousarray(qslice.T).astype(NP_DT),
                "kT": kT_cache[b],
                "v": v_cache[b],
                "fw": fw_host,
                "fb": fb_host,
            }
        )
    return in_maps


def assemble_out(results):
    out = np.empty((B, S, D), np.float32)
    for c in range(N_CORES):
        b, half = c // 2, c % 2
        out[b, half * M : (half + 1) * M, :] = results[c]["out"]
    return out


def kernel(query, key, value, fc_w, fc_b, _trace=False, _trace_kwargs=None):
    nc = _get_module()
    in_maps = make_in_maps(query, key, value, fc_w, fc_b)
    res = run_bass_kernel_spmd(
        nc,
        in_maps,
        core_ids=list(range(N_CORES)),
        trace=_trace,
        **(_trace_kwargs or {}),
    )
    out = assemble_out(res.results)
    if _trace:
        kernel.last_results = res
    return out


if __name__ == "__main__":
    rng = np.random.default_rng(0)
    q = rng.standard_normal((B, S, D)).astype(np.float32)
    k = rng.standard_normal((B, S, D)).astype(np.float32)
    v = rng.standard_normal((B, S, D)).astype(np.float32)
    w = (rng.standard_normal((D, D)) * 0.03).astype(np.float32)
    bvec = (rng.standard_normal((D,)) * 0.03).astype(np.float32)
    o = kernel(q, k, v, w, bvec)
    print("ran, out shape", o.shape)


# revision 40
# speedup vs baseline: 1.0101x; 1.0101x over previous
"""Trainium2 Bass kernel for nn_MultiHeadAttention_28260884808093.

MHA without QKV projections: heads formed by reshaping inputs directly,
scores scaled by 1/head_dim (not sqrt), softmax, attn@V, then fc_out.

Sharding: 8 cores = (batch, seq-half). Each core owns a disjoint
[1024, 1024] slice of the final output, so no device collectives are
needed (fc_out mixes head dims, not tokens). Host pre-transposes
q/k/fc_w so every matmul contraction lands on the partition axis.

Loop nest: pair j (8) -> m-half mh (2) -> key chunk c (16), flattened
into 256 substeps. Per substep, per engine:
  PE   : QK^T for both heads (row-tiled, concurrent) + the PV
         accumulation lagged PV_LAG substeps (so the PE never waits on
         the exp) -> stays HAM-warm at 2.4 GHz.
  ACT  : exp for the even head (exact, table spline).
  DVE  : exp for the odd head via a Schraudolph bf16 bit-trick
         (one tensor_scalar: i16 = s*128/(ln2*64) + (16256-c), bits
         reinterpreted as bf16). Softmax normalization cancels the
         common-mode error; keeping whole heads pure keeps the residual
         small (measured ~1.0e-2 vs the 2e-2 gate).
  GPSIMD: ONLY partition_broadcast (one library resident -- any second
         gpsimd op type causes ~5us LOAD_LIB swaps per call).
PV accumulators are [65, 512] (1 PSUM bank) per (pair, mh, head), Z
rides as a ones-column in V. Normalize = DVE reciprocal_approx_fast on
the Z row (PSUM src) + gpsimd broadcast + DVE multiply out of PSUM.

All matmul weights are full 128x128 blocks (K zero-padded per head, V
zero-padded across columns): that enables the compiler's automatic
fast-weight-load path, which is what lets back-to-back N=512 bf16
matmuls issue at the 216ns streaming cadence instead of the ~380ns
isolated-matmul latency.
"""

import sys

sys.path.insert(0, "/opt/trn_rl_repo")

import ml_dtypes
import numpy as np
from contextlib import ExitStack

import concourse.bass as bass  # noqa: F401
import concourse.bacc as bacc
import concourse.tile as tile
from concourse import mybir
from concourse import bass_utils
from concourse.bass_utils import run_bass_kernel_spmd

B, S, D = 4, 2048, 1024
H, DH = 16, 64
N_CORES = 8
M = (B * S) // N_CORES  # 1024 query tokens per core
PAIRS = H // 2
NCH = S // 128  # 16 key chunks of 128
NSUB = PAIRS * 2 * NCH  # 256 substeps: (pair, m-half, chunk)
PV_LAG = 6  # substeps PV trails QK/exp

F32 = mybir.dt.float32
BF16 = mybir.dt.bfloat16
FP8 = mybir.dt.float8e4
I16 = mybir.dt.int16
NP_DT = ml_dtypes.bfloat16
NP_FP8 = ml_dtypes.float8_e4m3

# Schraudolph exp->bf16-bits constants (odd heads, DVE):
#   i16 = round(s * 2^7/(ln2*DH) + (127*2^7 - C)) ; bits = bf16
LN2 = float(np.log(2.0))
SCHRAU_C1 = 128.0 / (LN2 * DH)
SCHRAU_C = 8.0
SCHRAU_C2 = 127.0 * 128.0 - SCHRAU_C

# Pairs whose odd head also runs on ACT (engine balance: DVE carries the
# normalize ops, so it takes 7 of the 8 odd heads, not 8).
ACT_B_PAIRS = frozenset({3})


def _sub_idx(u):
    return u // 32, (u // 16) % 2, u % 16  # pair, m-half, chunk


def _mha_body(ctx, tc, qT, kT, v, fw, fb, out):
    nc = tc.nc
    sb = ctx.enter_context(tc.tile_pool(name="sb", bufs=1))
    ps = ctx.enter_context(tc.tile_pool(name="ps", bufs=1, space="PSUM"))

    # PE warm-up burst: dense dummy matmuls while the first DMAs land,
    # so the HAM clock-gate flips to 2.4 GHz before the real work starts.
    wtile = sb.tile([128, 512], BF16, name="wtile", tag="wtile")
    nc.vector.memset(wtile[:], 0.0)
    for wi in range(32):
        wps = ps.tile(
            [128, 512], F32, name="wps", tag=("stA" if wi % 2 == 0 else "stB"), bufs=2
        )
        nc.tensor.matmul(
            wps[:], lhsT=wtile[:, 0:128], rhs=wtile[:], start=True, stop=True
        )

    # ---- resident tensors (pair-0 critical loads first; fc weights at
    # the end so they don't delay the pipeline ramp) ----
    qt_sb, fw_sb, attn = [], [], []
    for j in range(8):
        qt = sb.tile([128, M], BF16, name=f"qt{j}", tag=f"qt{j}")
        nc.sync.dma_start(out=qt[:], in_=qT[j * 128 : (j + 1) * 128, :])
        qt_sb.append(qt)
        at = sb.tile([128, M], BF16, name=f"attn{j}", tag=f"attn{j}")
        attn.append(at)

    # v as [p, chunk, d] so one DMA per head loads all 16 chunks
    v_pcd = v.rearrange("(c p) d -> p c d", p=128)

    # K weights zero-padded to full 128 contraction partitions (full-
    # width LDWEIGHTS pipelines; narrow ones do not): head A in rows
    # 0:64 / zeros below, head B in rows 64:128 / zeros above.
    # Persistent rotating slots -> the zero halves are memset exactly once.
    ktA_slots, ktB_slots = [], []
    for s in range(3):
        ka = sb.tile([128, S], BF16, name=f"ktA{s}", tag=f"ktA{s}")
        nc.vector.memset(ka[64:128, :], 0.0)
        ktA_slots.append(ka)
        kb = sb.tile([128, S], BF16, name=f"ktB{s}", tag=f"ktB{s}")
        nc.vector.memset(kb[0:64, :], 0.0)
        ktB_slots.append(kb)

    # V weights at full 128 columns (enables the compiler's automatic
    # fast-weight-load): col 0 = ones (Z lands at PSUM partition 0,
    # 32-aligned for the custom reciprocal), cols 64:128 = v (PV rows
    # at partitions 64:128), cols 1:64 zero. Persistent rotating slots
    # so the constant columns are memset exactly once.
    va_slots, vb_slots = [], []
    for s in range(3):
        for nm, slots in (("va", va_slots), ("vb", vb_slots)):
            vt_ = sb.tile([128, NCH, 128], BF16, name=f"{nm}{s}", tag=f"{nm}{s}")
            nc.vector.memset(vt_[:, :, 0:1], 1.0)
            nc.vector.memset(vt_[:, :, 1:DH], 0.0)
            slots.append(vt_)

    kt_t, va_t, vb_t = {}, {}, {}

    def issue_pair_loads(j):
        ka = ktA_slots[j % 3]
        nc.sync.dma_start(out=ka[0:64, :], in_=kT[j * 128 : j * 128 + 64, :])
        kb = ktB_slots[j % 3]
        nc.sync.dma_start(out=kb[64:128, :], in_=kT[j * 128 + 64 : (j + 1) * 128, :])
        kt_t[j] = (ka, kb)
        va = va_slots[j % 3]
        nc.sync.dma_start(
            out=va[:, :, DH:128], in_=v_pcd[:, :, (2 * j) * DH : (2 * j + 1) * DH]
        )
        va_t[j] = va
        vb = vb_slots[j % 3]
        nc.sync.dma_start(
            out=vb[:, :, DH:128], in_=v_pcd[:, :, (2 * j + 1) * DH : (2 * j + 2) * DH]
        )
        vb_t[j] = vb

    issue_pair_loads(0)

    # fc weights + bias, needed only in the fc phase
    for j in range(8):
        fwt = sb.tile([128, D], BF16, name=f"fw{j}", tag=f"fw{j}")
        nc.sync.dma_start(out=fwt[:], in_=fw[j * 128 : (j + 1) * 128, :])
        fw_sb.append(fwt)
    fb_sb = sb.tile([1, D], F32, name="fb_sb", tag="fb")
    nc.sync.dma_start(out=fb_sb[:], in_=fb[0:1, :])
    fbb = sb.tile([128, D], F32, name="fbb", tag="fbb")
    nc.gpsimd.partition_broadcast(fbb[:], fb_sb[:], channels=128)

    po_tiles = {}
    pa_hist, pb_hist = {}, {}

    def emit_qk_exp(u):
        j, mh, c = _sub_idx(u)
        if mh == 0 and c == 0 and j + 1 < PAIRS:
            issue_pair_loads(j + 1)
        if c == 0:
            oA = ps.tile([128, 512], F32, name="oA", tag="po", bufs=4)
            oB = ps.tile([128, 512], F32, name="oB", tag="po", bufs=4)
            po_tiles[(j, mh)] = (oA, oB)
        ka, kb = kt_t[j]
        ms = slice(mh * 512, (mh + 1) * 512)
        sA = ps.tile([128, 512], F32, name="sA", tag="stA", bufs=2)
        sB = ps.tile([128, 512], F32, name="sB", tag="stB", bufs=2)
        nc.tensor.matmul(
            sA[:],
            lhsT=ka[:, c * 128 : (c + 1) * 128],
            rhs=qt_sb[j][:, ms],
            start=True,
            stop=True,
        )
        nc.tensor.matmul(
            sB[:],
            lhsT=kb[:, c * 128 : (c + 1) * 128],
            rhs=qt_sb[j][:, ms],
            start=True,
            stop=True,
        )
        pa = sb.tile([128, 512], BF16, name="pa", tag="pa", bufs=10)
        nc.scalar.activation(
            out=pa[:],
            in_=sA[:],
            func=mybir.ActivationFunctionType.Exp,
            bias=0.0,
            scale=1.0 / DH,
        )
        if j in ACT_B_PAIRS:
            pb = sb.tile([128, 512], BF16, name="pbx", tag="pb", bufs=10)
            nc.scalar.activation(
                out=pb[:],
                in_=sB[:],
                func=mybir.ActivationFunctionType.Exp,
                bias=0.0,
                scale=1.0 / DH,
            )
        else:
            pb = sb.tile([128, 512], I16, name="pb", tag="pb", bufs=10)
            nc.vector.tensor_scalar(
                out=pb[:],
                in0=sB[:],
                scalar1=SCHRAU_C1,
                scalar2=SCHRAU_C2,
                op0=mybir.AluOpType.mult,
                op1=mybir.AluOpType.add,
            )
        pa_hist[u] = pa
        pb_hist[u] = pb

    def emit_pv(u):
        j, mh, c = _sub_idx(u)
        oA, oB = po_tiles[(j, mh)]
        pa = pa_hist.pop(u)
        pb = pb_hist.pop(u)
        nc.tensor.matmul(
            oA[:],
            lhsT=va_t[j][:, c, :],
            rhs=pa[:],
            start=(c == 0),
            stop=(c == NCH - 1),
        )
        rhs_b = pb[:] if pb.dtype == BF16 else pb[:].bitcast(BF16)
        nc.tensor.matmul(
            oB[:],
            lhsT=vb_t[j][:, c, :],
            rhs=rhs_b,
            start=(c == 0),
            stop=(c == NCH - 1),
        )

    def group_finish_ops(j, mh):
        """Normalize ops for group (j, mh), interleaved one per substep.
        recip + mul on DVE (PSUM source), broadcast on gpsimd."""
        oA, oB = po_tiles.pop((j, mh))
        ms = slice(mh * 512, (mh + 1) * 512)
        ops = []
        for h, oX in ((0, oA), (1, oB)):
            po = h * 64
            rz = sb.tile([1, 512], F32, name="rz", tag="rz", bufs=4)
            zbc = sb.tile([64, 512], F32, name="zbc", tag="zbc", bufs=4)

            def f_recip(oX=oX, rz=rz):
                nc.vector.reciprocal_approx_fast(out=rz[:], in_=oX[0:1, :])

            def f_bcast(zbc=zbc, rz=rz):
                nc.gpsimd.partition_broadcast(zbc[:], rz[:], channels=64)

            def f_mul(oX=oX, zbc=zbc, j=j, po=po, ms=ms):
                nc.vector.tensor_mul(
                    attn[j][po : po + 64, ms], oX[DH:128, :], zbc[:]
                )

            ops += [f_recip, f_bcast, f_mul]
        # release both PSUM slots early: A-ops then B-recip before muls
        return [ops[0], ops[3], ops[1], ops[4], ops[2], ops[5]]

    pending = []
    for u in range(NSUB + PV_LAG):
        if u < NSUB:
            emit_qk_exp(u)
        if u >= PV_LAG:
            up = u - PV_LAG
            emit_pv(up)
            jp, mhp, cp = _sub_idx(up)
            if cp == NCH - 1:
                pending.extend(group_finish_ops(jp, mhp))
        if u < NSUB:
            for f in pending[:2]:
                f()
            del pending[:2]
    for f in pending:
        f()

    # ---- fc_out: out[m, o] = attn_T.T @ fw + b ----
    for mi in range(8):
        for s2 in range(2):
            os_ = slice(s2 * 512, (s2 + 1) * 512)
            pf = ps.tile(
                [128, 512], F32, name="pf", tag=("stA" if s2 == 0 else "stB"), bufs=2
            )
            for jj in range(8):
                nc.tensor.matmul(
                    pf[:],
                    lhsT=attn[jj][:, mi * 128 : (mi + 1) * 128],
                    rhs=fw_sb[jj][:, os_],
                    start=(jj == 0),
                    stop=(jj == 7),
                )
            ob = sb.tile([128, 512], F32, name="ob", tag="ob", bufs=4)
            nc.vector.tensor_add(ob[:], pf[:], fbb[:, os_])
            nc.sync.dma_start(out=out[mi * 128 : (mi + 1) * 128, os_], in_=ob[:])


def build_module():
    nc = bacc.Bacc("TRN2", target_bir_lowering=False, debug=False, num_devices=N_CORES)
    qT = nc.dram_tensor("qT", [D, M], BF16, kind="ExternalInput")
    kT = nc.dram_tensor("kT", [D, S], BF16, kind="ExternalInput")
    v = nc.dram_tensor("v", [S, D], BF16, kind="ExternalInput")
    fw = nc.dram_tensor("fw", [D, D], BF16, kind="ExternalInput")
    fb = nc.dram_tensor("fb", [1, D], F32, kind="ExternalInput")
    out = nc.dram_tensor("out", [M, D], F32, kind="ExternalOutput")
    with tile.TileContext(nc) as tc:
        with ExitStack() as ctx:
            _mha_body(ctx, tc, qT.ap(), kT.ap(), v.ap(), fw.ap(), fb.ap(), out.ap())
    nc.compile()
    return nc


_NC_CACHE = None


def _get_module():
    global _NC_CACHE
    if _NC_CACHE is None:
        _NC_CACHE = build_module()
    return _NC_CACHE


def make_in_maps(query, key, value, fc_w, fc_b):
    fw_host = np.ascontiguousarray(fc_w.T).astype(NP_DT)
    fb_host = np.ascontiguousarray(np.asarray(fc_b, np.float32).reshape(1, D))
    in_maps = []
    kT_cache, v_cache = {}, {}
    for c in range(N_CORES):
        b, half = c // 2, c % 2
        if b not in kT_cache:
            kT_cache[b] = np.ascontiguousarray(key[b].T).astype(NP_DT)
            v_cache[b] = np.ascontiguousarray(value[b]).astype(NP_DT)
        qslice = query[b, half * M : (half + 1) * M, :]
        in_maps.append(
            {
                "qT": np.ascontiguousarray(qslice.T).astype(NP_DT),
                "kT": kT_cache[b],
                "v": v_cache[b],
                "fw": fw_host,
                "fb": fb_host,
            }
        )
    return in_maps


def assemble_out(results):
    out = np.empty((B, S, D), np.float32)
    for c in range(N_CORES):
        b, half = c // 2, c % 2
        out[b, half * M : (half + 1) * M, :] = results[c]["out"]
    return out


def kernel(query, key, value, fc_w, fc_b, _trace=False, _trace_kwargs=None):
    nc = _get_module()
    in_maps = make_in_maps(query, key, value, fc_w, fc_b)
    res = run_bass_kernel_spmd(
        nc,
        in_maps,
        core_ids=list(range(N_CORES)),
        trace=_trace,
        **(_trace_kwargs or {}),
    )
    out = assemble_out(res.results)
    if _trace:
        kernel.last_results = res
    return out


if __name__ == "__main__":
    rng = np.random.default_rng(0)
    q = rng.standard_normal((B, S, D)).astype(np.float32)
    k = rng.standard_normal((B, S, D)).astype(np.float32)
    v = rng.standard_normal((B, S, D)).astype(np.float32)
    w = (rng.standard_normal((D, D)) * 0.03).astype(np.float32)
    bvec = (rng.standard_normal((D,)) * 0.03).astype(np.float32)
    o = kernel(q, k, v, w, bvec)
    print("ran, out shape", o.shape)
